# revision 28
# baseline (speedup 1.0000x reference)
"""GCN (3-layer GCNConv + mean-pool + MLP) on 8 Trainium2 NeuronCores — V2.

Strategy (graph/data parallel, per sharding hint):
  - Nodes partitioned by graph id into 8 contiguous slices; each core owns
    edges whose destination is local.  Aggregation = dma_gather of source
    rows + one-hot matmul accumulation into PSUM (one-hot weights carry the
    GCN edge normalization dis[src]*dis[dst], built on host).
  - NO self-loop gather slots: the diagonal term dis_i^2 * (h W) is added
    per destination group with one extra PE matmul, W^T @ (h * dis2)
    (layer 1: folded into the post-aggregation transform on x).  dis2 is
    partition-broadcast + DVE-multiplied per sg on the fly.
  - Layer 1 is aggregate-first: gathers read a host-built node-major padded
    x table (pure input), so they start at t=0; the W1 transform happens
    after aggregation on the local slice only.  No replicated transform-1,
    no 25.6MB table write.
  - Layers 2/3: the local transform (h^T W) and a CHUNKED AllGather are
    interleaved into the previous layer's aggregation at sg granularity
    (lag 1), so only the last AllGather chunk sits on the critical path.
  - Pooling (PE transpose + membership matmul) interleaved into layer-3
    aggregation the same way.

The gather path is HBM-bank-throughput-bound (~5ns/descriptor); descriptor
count = edge count is the floor, so everything else hides behind it.
"""
import os
import sys

sys.path.insert(0, "/opt/trn_rl_repo")

import numpy as np

from concourse import bacc, bass, mybir, tile
from concourse import library_config
from concourse.bass_utils import run_bass_kernel_spmd
from concourse.masks import make_identity

F32 = mybir.dt.float32
BF16 = mybir.dt.bfloat16
I16 = mybir.dt.int16
NP_BF16 = mybir.dt.np(BF16)

N_CORES = 8
HID = 128
N_CLASSES = 3

GROUP = int(os.environ.get("GNN_GROUP", "128"))      # dst nodes per PSUM group
SG_GROUPS = int(os.environ.get("GNN_SG", "4"))       # groups per gather call
NCH = int(os.environ.get("GNN_NCH", "5"))            # AllGather chunks
NQ = int(os.environ.get("GNN_NQ", "1"))              # SWDGE queues
RING = int(os.environ.get("GNN_RING", "16384"))

_BUILD_CACHE = {}
LAST_EXEC_NS = None


# ----------------------------------------------------------------- host prep

def _prep(x, edge_index, batch):
    N, F = x.shape
    G = int(batch.max()) + 1 if batch.size else 1

    assert G % N_CORES == 0, G
    gpc = G // N_CORES

    batch = np.asarray(batch, dtype=np.int64)
    src = np.asarray(edge_index[0], dtype=np.int64)
    dst = np.asarray(edge_index[1], dtype=np.int64)

    gstart = np.searchsorted(batch, np.arange(G + 1), side="left")
    starts = gstart[0 : G + 1 : gpc]            # [C+1] node boundaries
    M = np.diff(starts)                          # nodes per core
    unit = GROUP * SG_GROUPS * NCH               # sg width x AG chunks
    Mp = int(np.ceil(M.max() / unit) * unit)
    R = N_CORES * Mp                             # table rows
    assert R <= 131072, R
    assert Mp % 128 == 0
    NB = Mp // 128                               # local 128-node blocks

    # node -> table row, CHUNK-MAJOR: chunk k of every core is contiguous
    # ([nch, N_CORES, B, HID] layout) so each AllGather chunk output is a
    # contiguous row range (BIR requires contiguous collective outputs).
    Bc = Mp // NCH
    owner = np.searchsorted(starts, np.arange(N), side="right") - 1
    mloc_of = np.arange(N) - starts[owner]
    row_of = (mloc_of // Bc) * (N_CORES * Bc) + owner * Bc + (mloc_of % Bc)

    # degree (in-degree over real edges) + 1, as in PyG gcn_norm
    deg = np.bincount(dst, minlength=N).astype(np.float32) + 1.0
    dis = 1.0 / np.sqrt(deg)

    # gather index ranges (signed-base int16)
    if R <= 32768:
        bases = [0]
    else:
        bases = [32768, 98304]
    n_ranges = len(bases)

    ngroups = Mp // GROUP
    assert ngroups % SG_GROUPS == 0
    nsg = ngroups // SG_GROUPS
    nkeys = nsg * n_ranges * SG_GROUPS

    # per-core edge arrays (NO self loops): src table row, local dst, weight
    e_owner = owner[dst]
    core_rows, core_d, core_w = [], [], []
    for c in range(N_CORES):
        m = e_owner == c
        s_c = int(starts[c])
        esrc, edst = src[m], dst[m]
        core_rows.append(row_of[esrc])
        core_d.append(edst - s_c)
        core_w.append((dis[esrc] * dis[edst]).astype(np.float32))

    def edge_keys(rows, d):
        gidx = d // GROUP
        sg = gidx // SG_GROUPS
        g_in = gidx % SG_GROUPS
        if n_ranges == 1:
            r = np.zeros_like(rows)
        else:
            r = (rows >= 65536).astype(np.int64)
        return (sg * n_ranges + r) * SG_GROUPS + g_in

    # uniform chunk counts per key (max over cores)
    kc = np.zeros((N_CORES, nkeys), np.int64)
    core_keys = []
    for c in range(N_CORES):
        key = edge_keys(core_rows[c], core_d[c])
        core_keys.append(key)
        kc[c] = (np.bincount(key, minlength=nkeys) + 127) // 128
    K = kc.max(axis=0)
    T = int(K.sum())
    slot_off = np.zeros(nkeys + 1, np.int64)
    np.cumsum(K * 128, out=slot_off[1:])
    S = int(slot_off[-1])
    S16 = S // 16

    IDX = np.zeros((N_CORES, 128, S16), np.int16)
    OH = np.zeros((N_CORES, 128, T, GROUP), NP_BF16)
    for c in range(N_CORES):
        rows, d, w = core_rows[c], core_d[c], core_w[c]
        key = core_keys[c]
        order = np.argsort(key, kind="stable")
        ks = key[order]
        if len(ks):
            new = np.empty(len(ks), bool)
            new[0] = True
            new[1:] = ks[1:] != ks[:-1]
            run_start = np.nonzero(new)[0][np.cumsum(new) - 1]
            pos = np.arange(len(ks)) - run_start
        else:
            pos = np.zeros(0, np.int64)
        slot = slot_off[ks] + pos
        rows_o = rows[order]
        if n_ranges == 1:
            i16 = rows_o
        else:
            rr = rows_o >= 65536
            i16 = rows_o - np.where(rr, 98304, 32768)
        idx_slots = np.zeros(S, np.int16)
        idx_slots[slot] = i16.astype(np.int16)
        IDX[c] = np.tile(idx_slots.reshape(S16, 16).T, (8, 1))
        # dis-weighted one-hot, [128 part(slot%128), T chunk, GROUP col]
        ohc = np.zeros((128, T, GROUP), np.float32)
        p = slot % 128
        t = slot // 128
        col = d[order] % GROUP
        ohc[p, t, col] = w[order]
        OH[c] = ohc.astype(NP_BF16)

    # x node-major padded [R, 128], bf16 (cols F..127 zero, pad rows zero)
    X_NM = np.zeros((R, 128), np.float32)
    X_NM[row_of, :F] = np.asarray(x, np.float32)
    X_NM = X_NM.astype(NP_BF16)

    # per-core: x*dis2 feature-major [128, Mp]; dis2 row [1, Mp]
    XS_FM = np.zeros((N_CORES, 128, Mp), NP_BF16)
    D2R = np.zeros((N_CORES, 1, Mp), NP_BF16)
    for c in range(N_CORES):
        s_c, e_c = int(starts[c]), int(starts[c + 1])
        mloc = e_c - s_c
        xs = (np.asarray(x[s_c:e_c], np.float32)
              * (dis[s_c:e_c] ** 2)[:, None]).T
        XS_FM[c, :F, :mloc] = xs.astype(NP_BF16)
        D2R[c, 0, :mloc] = (dis[s_c:e_c] ** 2).astype(NP_BF16)

    # pooling matrix: GO[p, b, g] = 1/cnt_g if local node b*128+p in graph g
    GO = np.zeros((N_CORES, 128, NB, gpc), np.float32)
    for c in range(N_CORES):
        s_c, e_c = int(starts[c]), int(starts[c + 1])
        mloc = e_c - s_c
        bl = (batch[s_c:e_c] - c * gpc).astype(np.int64)
        cnt = np.bincount(bl, minlength=gpc).astype(np.float32)
        inv = 1.0 / np.maximum(cnt, 1.0)
        n = np.arange(mloc)
        GO[c, n % 128, n // 128, bl] = inv[bl]

    meta = dict(
        F=F, R=R, Mp=Mp, T=T, S16=S16, nsg=nsg, ngroups=ngroups, NB=NB,
        n_ranges=n_ranges, bases=tuple(bases), gpc=gpc,
        group=GROUP, sg_groups=SG_GROUPS, nch=NCH,
        K=tuple(int(v) for v in K),
        slot_off=tuple(int(v) for v in slot_off),
    )
    in_maps = []
    for c in range(N_CORES):
        in_maps.append({
            "x_nm": X_NM,
            "idx": IDX[c],
            "oh": OH[c],
            "go": GO[c],
            "xs_fm": XS_FM[c],
            "d2r": D2R[c],
        })
    return meta, in_maps


def _weight_inputs(inputs):
    """Weight/bias arrays shared by kernel() and test.py."""
    w4 = np.asarray(inputs["W4"], np.float32)
    b3 = np.asarray(inputs["b3"], np.float32)
    b4p = np.asarray(inputs["b4"], np.float32) + b3 @ w4
    return {
        "w1": np.asarray(inputs["W1"], np.float32).astype(NP_BF16),
        "w2": np.asarray(inputs["W2"], np.float32).astype(NP_BF16),
        "w3": np.asarray(inputs["W3"], np.float32).astype(NP_BF16),
        "w4": w4,
        "w5": np.asarray(inputs["W5"], np.float32),
        "b1": np.broadcast_to(np.asarray(inputs["b1"], np.float32)[:, None], (HID, 1)).copy(),
        "b2": np.broadcast_to(np.asarray(inputs["b2"], np.float32)[:, None], (HID, 1)).copy(),
        "b4p": b4p[None, :],
        "b5": np.asarray(inputs["b5"], np.float32)[None, :],
    }


# --------------------------------------------------------------- device build

def _build(meta):
    F = meta["F"]
    R = meta["R"]
    Mp = meta["Mp"]
    T = meta["T"]
    S16 = meta["S16"]
    nsg = meta["nsg"]
    NB = meta["NB"]
    n_ranges = meta["n_ranges"]
    bases = meta["bases"]
    gpc = meta["gpc"]
    group = meta["group"]
    sg_groups = meta["sg_groups"]
    nch = meta["nch"]
    K = np.array(meta["K"], np.int64).reshape(nsg, n_ranges, sg_groups)
    slot_off = np.array(meta["slot_off"], np.int64)

    assert nsg % nch == 0, (nsg, nch)
    sg_per_ch = nsg // nch
    sgw = sg_groups * group                      # columns per sg
    blk_per_sg = sgw // 128
    B = Mp // nch                                # cin chunk rows

    max_ch_r = [int(K[:, r, :].sum(axis=1).max()) for r in range(n_ranges)]
    max_ch_tot = int(K.sum(axis=(1, 2)).max())

    nc = bacc.Bacc("TRN2", num_swdge_queues=NQ, dynamic_dma_scratch_size=RING)

    x_nm = nc.declare_dram_parameter("x_nm", [R, 128], BF16, isOutput=False)
    idx_p = nc.declare_dram_parameter("idx", [128, S16], I16, isOutput=False)
    oh_p = nc.declare_dram_parameter("oh", [128, T, group], BF16, isOutput=False)
    go_p = nc.declare_dram_parameter("go", [128, NB, gpc], F32, isOutput=False)
    xs_p = nc.declare_dram_parameter("xs_fm", [128, Mp], BF16, isOutput=False)
    d2_p = nc.declare_dram_parameter("d2r", [1, Mp], BF16, isOutput=False)
    w1 = nc.declare_dram_parameter("w1", [F, HID], BF16, isOutput=False)
    w2 = nc.declare_dram_parameter("w2", [HID, HID], BF16, isOutput=False)
    w3 = nc.declare_dram_parameter("w3", [HID, HID], BF16, isOutput=False)
    w4 = nc.declare_dram_parameter("w4", [HID, HID // 2], F32, isOutput=False)
    w5 = nc.declare_dram_parameter("w5", [HID // 2, N_CLASSES], F32, isOutput=False)
    b1 = nc.declare_dram_parameter("b1", [HID, 1], F32, isOutput=False)
    b2 = nc.declare_dram_parameter("b2", [HID, 1], F32, isOutput=False)
    b4p = nc.declare_dram_parameter("b4p", [1, HID // 2], F32, isOutput=False)
    b5 = nc.declare_dram_parameter("b5", [1, N_CLASSES], F32, isOutput=False)
    out_p = nc.declare_dram_parameter("out", [N_CLASSES, gpc], F32, isOutput=True)

    with tile.TileContext(nc) as tc:
        nc.gpsimd.load_library(library_config.mlp)
        with (
            tc.tile_pool(name="const", bufs=1) as constp,
            tc.tile_pool(name="hbuf", bufs=2) as hpool,
            tc.tile_pool(name="gbuf", bufs=2) as gpool,
            tc.tile_pool(name="ohb", bufs=2) as ohpool,
            tc.tile_pool(name="idxb", bufs=2) as idxpool,
            tc.tile_pool(name="ev", bufs=2) as evpool,
            tc.tile_pool(name="stb", bufs=2) as stpool,
            tc.tile_pool(name="hsg", bufs=2) as hsgpool,
            tc.tile_pool(name="tpsum", bufs=2, space="PSUM") as tpsum,
            tc.tile_pool(name="gpsum", bufs=1, space="PSUM") as gpsum,
            tc.tile_pool(name="ppsum", bufs=1, space="PSUM") as ppsum,
            tc.tile_pool(name="dram", bufs=1, space="DRAM") as dramp,
        ):
            # ---- constants in SBUF
            w_t = {}
            for nm, p, shp, dt in (("w1", w1, [F, HID], BF16),
                                   ("w2", w2, [HID, HID], BF16),
                                   ("w3", w3, [HID, HID], BF16),
                                   ("w4", w4, [HID, HID // 2], F32),
                                   ("w5", w5, [HID // 2, N_CLASSES], F32)):
                w_t[nm] = constp.tile(shp, dt, tag=nm, name=nm)
                nc.sync.dma_start(out=w_t[nm][:], in_=p[:])
            b_t = {}
            for nm, p, shp in (("b1", b1, [HID, 1]), ("b2", b2, [HID, 1]),
                               ("b4p", b4p, [1, HID // 2]), ("b5", b5, [1, N_CLASSES])):
                b_t[nm] = constp.tile(shp, F32, tag=nm, name=nm)
                nc.sync.dma_start(out=b_t[nm][:], in_=p[:])
            go_t = constp.tile([128, NB, gpc], F32, tag="go", name="go")
            nc.sync.dma_start(out=go_t[:], in_=go_p[:])
            xs_t = constp.tile([128, Mp], BF16, tag="xs", name="xs")
            nc.sync.dma_start(out=xs_t[:], in_=xs_p[:])
            d2_t = constp.tile([1, Mp], BF16, tag="d2", name="d2")
            nc.sync.dma_start(out=d2_t[:], in_=d2_p[:])
            ones_t = constp.tile([1, max(gpc, 128)], F32)
            nc.vector.memset(ones_t[:], 1.0)
            ident = constp.tile([128, 128], BF16)
            make_identity(nc, ident[:])

            # DRAM tables for layers 2 and 3 (cin chunked for AG overlap)
            cin = [[dramp.tile([B, HID], BF16, tag=f"cin{l}_{k}",
                               name=f"cin{l}_{k}")
                    for k in range(nch)] for l in range(2)]
            cout = [dramp.tile([R, HID], BF16, tag=f"cout{l}",
                               name=f"cout{l}")
                    for l in range(2)]

            def gather_sg(tbl_src, sg):
                """Load idx/oh, gather this sg's chunks."""
                t0 = int(slot_off[sg * n_ranges * sg_groups]) // 128
                ch_tot = int(K[sg].sum())
                idx_t = idxpool.tile([128, max_ch_tot * 8], I16, tag="idx")
                nc.sync.dma_start(out=idx_t[:, : ch_tot * 8],
                                  in_=idx_p[:, t0 * 8 : (t0 + ch_tot) * 8])
                oh_t = ohpool.tile([128, max_ch_tot, group], BF16, tag="oh")
                nc.sync.dma_start(out=oh_t[:, :ch_tot, :],
                                  in_=oh_p[:, t0 : t0 + ch_tot, :])
                gbs = []
                off = 0
                for r in range(n_ranges):
                    ch_r = int(K[sg, r].sum())
                    if ch_r == 0:
                        gbs.append(None)
                        continue
                    gb = gpool.tile([128, max_ch_r[r], HID], BF16, tag=f"gb{r}")
                    nix = ch_r * 128
                    nc.gpsimd.dma_gather(
                        gb[:, :ch_r, :],
                        tbl_src[bases[r] :, :],
                        idx_t[:, off * 8 : off * 8 + nix // 16],
                        nix, nix, HID,
                        single_packet=False,
                        queue_num=0,
                    )
                    gbs.append(gb)
                    off += ch_r
                return gbs, oh_t

            def agg_sg(sg, gbs, oh_t, self_w, h_prev, evict_to):
                """One-hot matmuls (+ optional self term) into PSUM groups.

                evict_to = (h_out, bias_ap, relu) or None (caller evicts).
                Returns gps tiles.
                """
                gps = [gpsum.tile([128, group], F32, tag=f"gp{i}",
                                  name=f"gp{i}") for i in range(sg_groups)]
                started = [False] * sg_groups
                if self_w is not None:
                    # self term: W^T @ (h_prev * dis2) for this sg's columns
                    n0 = sg * sgw
                    d2g = hsgpool.tile([128, sgw], BF16, tag="d2g")
                    nc.gpsimd.partition_broadcast(
                        d2g[:], d2_t[:1, n0 : n0 + sgw])
                    hsg = hsgpool.tile([128, sgw], BF16, tag="hsg")
                    nc.vector.tensor_mul(hsg[:], h_prev[:, n0 : n0 + sgw],
                                         d2g[:])
                    for g in range(sg_groups):
                        nc.tensor.matmul(
                            gps[g][:], lhsT=self_w,
                            rhs=hsg[:, g * group : (g + 1) * group],
                            start=True,
                            stop=int(K[sg, :, g].sum()) == 0)
                        started[g] = True
                remaining = [int(K[sg, :, g].sum()) for g in range(sg_groups)]
                ch = 0
                for r in range(n_ranges):
                    gch = 0
                    for g in range(sg_groups):
                        for t in range(int(K[sg, r, g])):
                            remaining[g] -= 1
                            nc.tensor.matmul(
                                gps[g][:],
                                lhsT=gbs[r][:, gch, :], rhs=oh_t[:, ch, :],
                                start=not started[g],
                                stop=remaining[g] == 0,
                            )
                            started[g] = True
                            ch += 1
                            gch += 1
                if evict_to is not None:
                    h_out, bias_ap, relu = evict_to
                    for g in range(sg_groups):
                        n0 = sg * sgw + g * group
                        nc.scalar.activation(
                            out=h_out[:, n0 : n0 + group],
                            in_=gps[g][:],
                            func=(mybir.ActivationFunctionType.Relu if relu
                                  else mybir.ActivationFunctionType.Copy),
                            **({"bias": bias_ap} if bias_ap is not None
                               else {}))
                return gps

            def transform_blocks(h_src, W, lay, sg):
                """cin transform+store for sg's blocks; AG chunk at chunk end."""
                st = stpool.tile([128, blk_per_sg, HID], BF16, tag="st")
                for j in range(blk_per_sg):
                    b = sg * blk_per_sg + j
                    tp = tpsum.tile([128, 128], F32, tag="tp", name="tp")
                    nc.tensor.matmul(tp[:], lhsT=h_src[:, b * 128 : (b + 1) * 128],
                                     rhs=W, start=True, stop=True)
                    nc.scalar.activation(out=st[:, j, :], in_=tp[:],
                                         func=mybir.ActivationFunctionType.Copy)
                k = sg // sg_per_ch
                r0 = (sg % sg_per_ch) * blk_per_sg * 128
                nc.sync.dma_start(
                    out=cin[lay][k][r0 : r0 + blk_per_sg * 128, :]
                    .rearrange("(j p) f -> p j f", p=128),
                    in_=st[:],
                )
                if sg % sg_per_ch == sg_per_ch - 1:
                    # chunk-major table: chunk k's rows are contiguous
                    nc.gpsimd.collective_compute(
                        "AllGather", mybir.AluOpType.bypass,
                        replica_groups=[list(range(N_CORES))],
                        ins=[cin[lay][k][:]],
                        outs=[cout[lay][k * N_CORES * B : (k + 1) * N_CORES * B, :]],
                    )

            def pool_blocks(h_src, pool_ps, sg):
                for j in range(blk_per_sg):
                    b = sg * blk_per_sg + j
                    tp = ppsum.tile([128, 128], BF16, tag="tp", name="ptp")
                    nc.tensor.transpose(out=tp[:],
                                        in_=h_src[:, b * 128 : (b + 1) * 128],
                                        identity=ident[:])
                    hnm = evpool.tile([128, 128], F32, tag="hnm")
                    nc.scalar.activation(out=hnm[:], in_=tp[:],
                                         func=mybir.ActivationFunctionType.Copy)
                    nc.tensor.matmul(pool_ps[:], lhsT=hnm[:], rhs=go_t[:, b, :],
                                     start=(b == 0), stop=(b == NB - 1))

            for _rep in range(int(os.environ.get("GNN_REPS", "1"))):
                # ---------------- layer 1: aggregate x, transform, feed L2
                h1 = hpool.tile([128, Mp], BF16, tag="h")
                for sg in range(nsg):
                    gbs, oh_t = gather_sg(x_nm, sg)
                    gps = agg_sg(sg, gbs, oh_t, None, None, None)
                    # h1 = relu(W1^T(aggx + x*dis2) + b1), per group
                    for g in range(sg_groups):
                        n0 = sg * sgw + g * group
                        aggx = evpool.tile([128, group], BF16, tag="aggx")
                        if int(K[sg, :, g].sum()) == 0:
                            nc.vector.memset(aggx[:], 0.0)
                        else:
                            nc.scalar.activation(
                                out=aggx[:], in_=gps[g][:],
                                func=mybir.ActivationFunctionType.Copy)
                        tp = tpsum.tile([128, 128], F32, tag="tp", name="tp")
                        nc.tensor.matmul(tp[:], lhsT=w_t["w1"][:],
                                         rhs=aggx[:F, :], start=True,
                                         stop=False)
                        nc.tensor.matmul(tp[:], lhsT=w_t["w1"][:],
                                         rhs=xs_t[:F, n0 : n0 + group],
                                         start=False, stop=True)
                        nc.scalar.activation(
                            out=h1[:, n0 : n0 + group], in_=tp[:],
                            func=mybir.ActivationFunctionType.Relu,
                            bias=b_t["b1"][:, :])
                    if sg > 0:
                        transform_blocks(h1, w_t["w2"][:], 0, sg - 1)
                transform_blocks(h1, w_t["w2"][:], 0, nsg - 1)

                # ---------------- layer 2
                h2 = hpool.tile([128, Mp], BF16, tag="h")
                for sg in range(nsg):
                    gbs, oh_t = gather_sg(cout[0], sg)
                    agg_sg(sg, gbs, oh_t, w_t["w2"][:], h1,
                           (h2, b_t["b2"][:, :], True))
                    if sg > 0:
                        transform_blocks(h2, w_t["w3"][:], 1, sg - 1)
                transform_blocks(h2, w_t["w3"][:], 1, nsg - 1)

                # ---------------- layer 3 + pooling
                h3 = hpool.tile([128, Mp], BF16, tag="h")
                pool_ps = ppsum.tile([128, gpc], F32, tag="pp", name="pool_ps")
                for sg in range(nsg):
                    gbs, oh_t = gather_sg(cout[1], sg)
                    agg_sg(sg, gbs, oh_t, w_t["w3"][:], h2, (h3, None, False))
                    if sg > 0:
                        pool_blocks(h3, pool_ps, sg - 1)
                pool_blocks(h3, pool_ps, nsg - 1)

                # ---------------- MLP
                pooled = evpool.tile([128, gpc], F32, tag="pooled")
                nc.scalar.activation(out=pooled[:], in_=pool_ps[:],
                                     func=mybir.ActivationFunctionType.Copy)
                zps = ppsum.tile([HID // 2, gpc], F32, tag="pp", name="zps")
                nc.tensor.matmul(zps[:], lhsT=w_t["w4"][:], rhs=pooled[:],
                                 start=True, stop=False)
                nc.tensor.matmul(zps[:], lhsT=b_t["b4p"][:], rhs=ones_t[:, :gpc],
                                 start=False, stop=True)
                z_t = evpool.tile([HID // 2, gpc], F32, tag="z")
                nc.scalar.activation(out=z_t[:], in_=zps[:],
                                     func=mybir.ActivationFunctionType.Relu)
                ops = ppsum.tile([N_CLASSES, gpc], F32, tag="pp", name="ops")
                nc.tensor.matmul(ops[:], lhsT=w_t["w5"][:], rhs=z_t[:],
                                 start=True, stop=False)
                nc.tensor.matmul(ops[:], lhsT=b_t["b5"][:], rhs=ones_t[:, :gpc],
                                 start=False, stop=True)
                o_t = evpool.tile([N_CLASSES, gpc], F32, tag="o")
                nc.scalar.activation(out=o_t[:], in_=ops[:],
                                     func=mybir.ActivationFunctionType.Copy)
                nc.sync.dma_start(out=out_p[:], in_=o_t[:])

    nc.compile()
    return nc


# -------------------------------------------------------------------- kernel

def kernel(**inputs):
    x = np.asarray(inputs["x"], np.float32)
    edge_index = np.asarray(inputs["edge_index"])
    batch = np.asarray(inputs["batch"])
    meta, in_maps = _prep(x, edge_index, batch)

    key = repr(sorted(meta.items()))
    if key not in _BUILD_CACHE:
        _BUILD_CACHE[key] = _build(meta)
    nc = _BUILD_CACHE[key]

    wmap = _weight_inputs(inputs)
    for im in in_maps:
        im.update(wmap)

    res = run_bass_kernel_spmd(nc, in_maps, list(range(N_CORES)))
    global LAST_EXEC_NS
    LAST_EXEC_NS = res.exec_time_ns
    gpc = meta["gpc"]
    G = gpc * N_CORES
    out = np.zeros((G, N_CLASSES), np.float32)
    for c in range(N_CORES):
        out[c * gpc : (c + 1) * gpc, :] = res.results[c]["out"].T
    return out
